# revision 18
# baseline (speedup 1.0000x reference)
"""Trainium2 Bass kernel for DelayedAgg GNN message passing.

Per batch b (one NeuronCore per batch, B=8 across 8 cores):
    xin  = concat(features[b], support_xyz[b].T)          # [67, N]
    x1   = relu(W1 @ xin + b1)                            # [128, N]
    x2nb = W2 @ x1                                        # [256, N]  (NO bias)
    out[c, m] = relu(max_k x2nb[c, idx[m,k]] + b2[c])
where bias2 + relu2 commute out of the neighbor max:
    max_k relu(W2 h_k + b2) = relu(max_k (W2 h_k) + b2)
so the device computes only max_k of bias-free conv2 rows; the +b2/relu/
transpose run on the host over the small [6000, 256] result.

Device plan:
  Phase 1: stream 512-column tiles: conv1 on PE (lhsT=W1^T stationary),
    ReLU+bias on ACT, then conv2 *transposed* via PE with lhsT=x1-chunk
    (output partition dim = point index) -> x2^T rows, cast bf16, DMA to
    a DRAM scratch x2t [24064, 256] bf16 (row j = point j's 256 channels).
  Phase 2: per m-tile of T=128 queries, one InstDMAGatherAnt (custom
    SWDGE ucode, transpose mode) gathers the 128*32 neighbor rows of
    x2t, landing channels on partitions: dst[p, c2, k*T+ml] =
    x2t[idx[ml,k], c2*128+p] as [128, 2, 4096] bf16. Then an in-place
    pairwise max tree over k (32->16->8->4->2->1) on DVE/ACT (bf16
    step-1 slices hit the 2x perf mode); the last level upcasts f32
    into an 8-tile staging buffer DMA'd to outT, which is already
    channel-major [256, 6016].
    (Tried and rejected: cce_op=max accumulate during DMA - the
    neuronxcc verifier only allows add; indirect_dma_start - walrus
    unrolls dynamic APs into per-index DMAs, ~30x too slow and wrong
    for >1 index per partition.)
"""

import os
import sys

import numpy as np

try:
    import concourse.bass as bass  # noqa: F401
except ImportError:  # pragma: no cover - container default path
    sys.path.insert(0, "/opt/trn_rl_repo")

import concourse.bass as bass
import concourse.bacc as bacc
import concourse.tile as tile
from concourse import mybir
from concourse.bass_utils import run_bass_kernel_spmd

import ml_dtypes

# Problem shapes (hardcoded per spec nn_DelayedAgg_76690936037739)
B = 8
N = 24000
M = 6000
K = 32
CIN = 64
CMID = 128
COUT = 256

NPAD = 24064          # 188 * 128 = 47 * 512
NT = NPAD // 512      # 47 supertiles of 512 support points
MT = 47               # m-tiles of 128 queries
MPAD = MT * 128       # 6016
STG = 8               # m-tiles per output staging buffer

FP32 = mybir.dt.float32
BF16 = mybir.dt.bfloat16
INT32 = mybir.dt.int32


def build_body(ctx, tc, xin, w1t, w2t, b1c, idxt, outT, x2t):
    nc = tc.nc

    singles = ctx.enter_context(tc.tile_pool(name="singles", bufs=1))
    w1t_sb = singles.tile([CIN + 3, CMID], FP32)
    nc.sync.dma_start(out=w1t_sb[:], in_=w1t.ap())
    w2t_sb = singles.tile([CMID, COUT], FP32)
    nc.sync.dma_start(out=w2t_sb[:], in_=w2t.ap())
    b1_sb = singles.tile([CMID, 1], FP32)
    nc.sync.dma_start(out=b1_sb[:], in_=b1c.ap())
    idx_sb = singles.tile([128, MT * (K * 128 // 16)], mybir.dt.int16)
    nc.sync.dma_start(out=idx_sb[:], in_=idxt.ap())

    xin_pool = ctx.enter_context(tc.tile_pool(name="xin", bufs=3))
    ps1_pool = ctx.enter_context(tc.tile_pool(name="ps1", bufs=2, space="PSUM"))
    x1_pool = ctx.enter_context(tc.tile_pool(name="x1", bufs=3))
    ps2_pool = ctx.enter_context(tc.tile_pool(name="ps2", bufs=4, space="PSUM"))
    stage_pool = ctx.enter_context(tc.tile_pool(name="stage", bufs=3))

    xin_ap = xin.ap()                                       # [67, NPAD]
    # x2t rows (4i+j)*128 + p <- stage[p, j, :] for supertile i
    x2t_v = x2t.ap().rearrange("(i t p) c -> i p t c", t=4, p=128)  # [47,128,4,256]

    relu = mybir.ActivationFunctionType.Relu
    for i in range(NT):
        xin_sb = xin_pool.tile([CIN + 3, 512], FP32)
        nc.sync.dma_start(out=xin_sb[:], in_=xin_ap[:, i * 512:(i + 1) * 512])
        ps1 = ps1_pool.tile([CMID, 512], FP32)
        nc.tensor.matmul(ps1[:], lhsT=w1t_sb[:], rhs=xin_sb[:], start=True, stop=True)
        x1_sb = x1_pool.tile([CMID, 512], FP32)
        nc.scalar.activation(x1_sb[:], ps1[:], relu, bias=b1_sb[:])
        stage = stage_pool.tile([128, 4, COUT], BF16)
        for j in range(4):
            ps2 = ps2_pool.tile([128, COUT], FP32)
            nc.tensor.matmul(
                ps2[:],
                lhsT=x1_sb[:, j * 128:(j + 1) * 128],
                rhs=w2t_sb[:],
                start=True,
                stop=True,
            )
            # f32 PSUM -> bf16 SBUF cast; alternate engines to balance load
            if j % 2 == 0:
                nc.vector.tensor_copy(stage[:, j, :], ps2[:])
            else:
                nc.scalar.activation(
                    stage[:, j, :], ps2[:], mybir.ActivationFunctionType.Copy
                )
        nc.sync.dma_start(out=x2t_v[i], in_=stage[:])

    # Phase boundary: gathers must observe every x2t row.
    tc.strict_bb_all_engine_barrier()

    T = 128                    # queries per gather call
    NI = K * T                 # 4096 indices per call
    NW = NI // 16              # idx words per partition (16-partition wrap)
    dst_pool = ctx.enter_context(tc.tile_pool(name="dst", bufs=3))
    out_pool = ctx.enter_context(tc.tile_pool(name="ostage", bufs=2))
    # outT is channel-major [256, MPAD]; channel c = c2*128 + p
    outT_v = outT.ap().rearrange("(c2 p) m -> p c2 m", p=128)  # [128, 2, MPAD]
    stage = None
    mx = mybir.AluOpType.max
    for t in range(MT):
        dst = dst_pool.tile([128, 2, NI], BF16)
        nc.gpsimd.dma_gather(
            out_ap=dst[:],
            in_ap=x2t.ap(),
            idxs_ap=idx_sb[:, t * NW:(t + 1) * NW],
            num_idxs=NI,
            num_idxs_reg=NI,
            elem_size=COUT,
            transpose=True,
            # default single_packet=True coalesces the whole stream into one
            # SDMA packet; the HW packet ceiling is ~64 descriptors, so at
            # 4096 idxs the engine crashes (NRT_EXEC_UNIT_UNRECOVERABLE).
            single_packet=False,
        )
        dk = dst[:].rearrange("p c2 (k m) -> p c2 k m", k=K)  # [128,2,32,T]
        # in-place pairwise max tree over the 32 neighbors
        for h in (16, 8, 4, 2):
            nc.any.tensor_tensor(
                out=dk[:, :, :h, :], in0=dk[:, :, :h, :], in1=dk[:, :, h:2 * h, :],
                op=mx,
            )
        g, r = divmod(t, STG)
        if r == 0:
            n_in_g = min(STG, MT - g * STG)
            stage = out_pool.tile([128, 2, STG * T], FP32)
        # final level + f32 upcast
        nc.any.tensor_tensor(
            out=stage[:, :, r * T:(r + 1) * T],
            in0=dk[:, :, 0, :], in1=dk[:, :, 1, :], op=mx,
        )
        if r == n_in_g - 1:
            nc.sync.dma_start(
                out=outT_v[:, :, g * STG * T:(g * STG + n_in_g) * T],
                in_=stage[:, :, :n_in_g * T],
            )


def build_program():
    nc = bacc.Bacc("TRN2", target_bir_lowering=False, debug=False)
    xin = nc.dram_tensor("xin", [CIN + 3, NPAD], FP32, kind="ExternalInput")
    w1t = nc.dram_tensor("w1t", [CIN + 3, CMID], FP32, kind="ExternalInput")
    w2t = nc.dram_tensor("w2t", [CMID, COUT], FP32, kind="ExternalInput")
    b1c = nc.dram_tensor("b1c", [CMID, 1], FP32, kind="ExternalInput")
    idxt = nc.dram_tensor(
        "idxt", [128, MT * (K * 128 // 16)], mybir.dt.int16, kind="ExternalInput"
    )
    outT = nc.dram_tensor("outT", [COUT, MPAD], FP32, kind="ExternalOutput")
    x2t = nc.dram_tensor("x2t", [NPAD, COUT], BF16, kind="Internal")

    from contextlib import ExitStack

    with tile.TileContext(nc) as tc:
        with ExitStack() as ctx:
            build_body(ctx, tc, xin, w1t, w2t, b1c, idxt, outT, x2t)
    nc.compile()
    return nc


def make_in_map(features_b, xyz_b, idx_b, W1, W2, b1):
    """Build one core's input map from one batch's raw inputs (numpy)."""
    xin = np.zeros((CIN + 3, NPAD), np.float32)
    xin[:CIN, :N] = features_b
    xin[CIN:CIN + 3, :N] = xyz_b.T
    idxp = np.zeros((MPAD, K), np.int16)
    idxp[:M] = idx_b.astype(np.int16)
    # per call t the index list is i = k*128 + ml -> neighbor[t*128+ml, k],
    # wrapped into 16 partitions (idx i at [i%16, i//16]) and replicated
    # across the 8 gpsimd cores' partition groups.
    NW = K * 128 // 16
    lst = idxp.reshape(MT, 128, K).transpose(0, 2, 1).reshape(MT, K * 128)
    w = lst.reshape(MT, NW, 16).transpose(0, 2, 1)      # [MT, 16, NW]
    w = np.tile(w, (1, 8, 1))                           # [MT, 128, NW]
    idxt = np.ascontiguousarray(
        w.transpose(1, 0, 2).reshape(128, MT * NW).astype(np.int16)
    )
    return {
        "xin": xin,
        "w1t": np.ascontiguousarray(W1.T.astype(np.float32)),
        "w2t": np.ascontiguousarray(W2.T.astype(np.float32)),
        "b1c": np.ascontiguousarray(b1.astype(np.float32).reshape(CMID, 1)),
        "idxt": idxt,
    }


def postprocess(outT_np, b2):
    """[256, MPAD] device result -> [256, 6000] final (adds b2, relu)."""
    o = outT_np[:, :M].astype(np.float32) + b2.astype(np.float32)[:, None]
    np.maximum(o, 0.0, out=o)
    return o


_CACHE = {}


def run(inputs, trace=False, **spmd_kwargs):
    """Run on 8 NeuronCores; returns (out [8,256,6000] f32, BassKernelResults)."""
    features = np.asarray(inputs["features"], np.float32)
    support_xyz = np.asarray(inputs["support_xyz"], np.float32)
    neighbor_idx = np.asarray(inputs["neighbor_idx"])
    W1 = np.asarray(inputs["W1"], np.float32)
    W2 = np.asarray(inputs["W2"], np.float32)
    b1 = np.asarray(inputs["b1"], np.float32)
    b2 = np.asarray(inputs["b2"], np.float32)

    if "nc" not in _CACHE:
        _CACHE["nc"] = build_program()
    nc = _CACHE["nc"]

    in_maps = [
        make_in_map(features[b], support_xyz[b], neighbor_idx[b], W1, W2, b1)
        for b in range(B)
    ]
    res = run_bass_kernel_spmd(
        nc, in_maps, core_ids=list(range(B)), trace=trace, **spmd_kwargs
    )
    out = np.stack(
        [postprocess(res.results[b]["outT"], b2) for b in range(B)]
    ).astype(np.float32)
    return out, res


def kernel(query_xyz, support_xyz, features, neighbor_idx, W1, b1, W2, b2,
           **unused):
    del query_xyz  # neighborhoods are precomputed; query coords unused
    out, _ = run(
        dict(
            support_xyz=support_xyz,
            features=features,
            neighbor_idx=neighbor_idx,
            W1=W1,
            b1=b1,
            W2=W2,
            b2=b2,
        )
    )
    return out


if __name__ == "__main__":
    # Smoke: build only
    nc = build_program()
    print("program built ok")


# revision 19
# speedup vs baseline: 2.5146x; 2.5146x over previous
"""Trainium2 Bass kernel for DelayedAgg GNN message passing.

Per batch b (one NeuronCore per batch, B=8 across 8 cores):
    xin  = concat(features[b], support_xyz[b].T)          # [67, N]
    x1   = relu(W1 @ xin + b1)                            # [128, N]
    x2nb = W2 @ x1                                        # [256, N]  (NO bias)
    out[c, m] = relu(max_k x2nb[c, idx[m,k]] + b2[c])
bias2 + relu2 commute out of the neighbor max
(max_k relu(W2 h_k + b2) = relu(max_k (W2 h_k) + b2)), so the device
computes only max_k of bias-free conv2 rows; +b2/relu/transpose run on
the host over the small [6000, 256] result.

Device plan:
  Phase 1 (12 supertiles of 2048 support points): conv1 on PE
    (lhsT=W1^T stationary), ReLU+bias on ACT, then conv2 *transposed*
    via PE with lhsT=x1-chunk (output partition = point index) ->
    x2^T rows, cast bf16, DMA (1 MB writes, alternating between the
    two HWDGE queues) to DRAM scratch x2t [24576, 256] bf16.
  Phase 2 (47 m-tiles of 128 queries): one InstDMAGatherAnt (custom
    SWDGE ucode) per tile gathers its 128*32 neighbor rows:
    dst[ml, k, :] = x2t[idx[ml,k], :] as [128, 32, 256] bf16. Gathers
    round-robin over 4 SWDGE queues (4 Q7 core pairs generate
    descriptors in parallel; descgen is the dominant cost at ~17-34us
    per 4096-row call on one pair). An in-place pairwise max tree over
    k (32->16->8->4->2->1) runs on flat unit-stride 2D slices (bf16
    2x DVE mode); the last level upcasts f32 into an 8-tile staging
    buffer DMA'd to outT [6016, 256] (query-major; host transposes).
  Pitfalls baked in: single_packet=False (the default coalesces all
    descriptors into one SDMA packet; HW caps packets at ~64
    descriptors -> NRT_EXEC_UNIT_UNRECOVERABLE); cce_op=max and
    indirect_dma_start both rejected (verifier only allows add;
    walrus unrolls dynamic APs into per-index DMAs).
"""

import os
import sys

import numpy as np

try:
    import concourse.bass as bass  # noqa: F401
except ImportError:  # pragma: no cover - container default path
    sys.path.insert(0, "/opt/trn_rl_repo")

import concourse.bass as bass
import concourse.bacc as bacc
import concourse.tile as tile
from concourse import mybir
from concourse.bass_utils import run_bass_kernel_spmd

import ml_dtypes

# Problem shapes (hardcoded per spec nn_DelayedAgg_76690936037739)
B = 8
N = 24000
M = 6000
K = 32
CIN = 64
CMID = 128
COUT = 256

SUP = 2048            # support points per phase-1 supertile
NPAD = 24576          # 12 * 2048 = 192 * 128
NSUP = NPAD // SUP    # 12
MT = 47               # m-tiles of 128 queries
MPAD = MT * 128       # 6016
STG = 8               # m-tiles per output staging buffer
T = 128               # queries per gather call
NI = K * T            # 4096 indices per call
NW = NI // 16         # idx int16 words per partition per call
NQ = 4                # SWDGE queues for gather descgen

FP32 = mybir.dt.float32
BF16 = mybir.dt.bfloat16

_CACHE = {}


def build_body(ctx, tc, xin, w1t, w2t, b1c, idxt, outT, x2t):
    nc = tc.nc

    singles = ctx.enter_context(tc.tile_pool(name="singles", bufs=1))
    w1t_sb = singles.tile([CIN + 3, CMID], FP32)
    nc.sync.dma_start(out=w1t_sb[:], in_=w1t.ap())
    w2t_sb = singles.tile([CMID, COUT], FP32)
    nc.sync.dma_start(out=w2t_sb[:], in_=w2t.ap())
    b1_sb = singles.tile([CMID, 1], FP32)
    nc.sync.dma_start(out=b1_sb[:], in_=b1c.ap())
    idx_sb = singles.tile([128, MT * NW], mybir.dt.int16)
    nc.sync.dma_start(out=idx_sb[:], in_=idxt.ap())

    xin_pool = ctx.enter_context(tc.tile_pool(name="xin", bufs=2))
    ps1_pool = ctx.enter_context(tc.tile_pool(name="ps1", bufs=2, space="PSUM"))
    x1_pool = ctx.enter_context(tc.tile_pool(name="x1", bufs=2))
    ps2_pool = ctx.enter_context(tc.tile_pool(name="ps2", bufs=4, space="PSUM"))
    stage_pool = ctx.enter_context(tc.tile_pool(name="stage", bufs=2))

    xin_ap = xin.ap()                                       # [67, NPAD]
    # x2t row (i*16 + j)*128 + p  <-  stage[p, j, :] for supertile i
    x2t_v = x2t.ap().rearrange("(i t p) c -> i p t c", t=SUP // 128, p=128)

    relu = mybir.ActivationFunctionType.Relu
    cpy = mybir.ActivationFunctionType.Copy
    for i in range(NSUP):
        xin_sb = xin_pool.tile([CIN + 3, SUP], FP32)
        nc.sync.dma_start(out=xin_sb[:], in_=xin_ap[:, i * SUP:(i + 1) * SUP])
        x1_sb = x1_pool.tile([CMID, SUP], FP32)
        for q in range(SUP // 512):
            ps1 = ps1_pool.tile([CMID, 512], FP32)
            nc.tensor.matmul(
                ps1[:], lhsT=w1t_sb[:], rhs=xin_sb[:, q * 512:(q + 1) * 512],
                start=True, stop=True,
            )
            nc.scalar.activation(
                x1_sb[:, q * 512:(q + 1) * 512], ps1[:], relu, bias=b1_sb[:]
            )
        stage = stage_pool.tile([128, SUP // 128, COUT], BF16)
        for j in range(SUP // 128):
            ps2 = ps2_pool.tile([128, COUT], FP32)
            nc.tensor.matmul(
                ps2[:], lhsT=x1_sb[:, j * 128:(j + 1) * 128], rhs=w2t_sb[:],
                start=True, stop=True,
            )
            # f32 PSUM -> bf16 SBUF cast; alternate engines to balance
            if j % 2 == 0:
                nc.vector.tensor_copy(stage[:, j, :], ps2[:])
            else:
                nc.scalar.activation(stage[:, j, :], ps2[:], cpy)
        # 1 MB write; alternate the two HWDGE queues (SP / ACT)
        eng = nc.sync if i % 2 == 0 else nc.scalar
        eng.dma_start(out=x2t_v[i], in_=stage[:])

    # Phase boundary: gathers must observe every x2t row.
    tc.strict_bb_all_engine_barrier()

    dst_pool = ctx.enter_context(tc.tile_pool(name="dst", bufs=4))
    out_pool = ctx.enter_context(tc.tile_pool(name="ostage", bufs=2))
    outT_v = outT.ap().rearrange("(t p) c -> t p c", p=128)  # [47, 128, 256]
    stage2 = None
    mx = mybir.AluOpType.max
    for t in range(MT):
        dst = dst_pool.tile([128, K, COUT], BF16)
        nc.gpsimd.dma_gather(
            out_ap=dst[:],
            in_ap=x2t.ap(),
            idxs_ap=idx_sb[:, t * NW:(t + 1) * NW],
            num_idxs=NI,
            num_idxs_reg=NI,
            elem_size=COUT,
            transpose=False,
            single_packet=False,
            queue_num=t % NQ,
        )
        # in-place pairwise max tree over k; flat unit-stride slices
        dv = dst[:].rearrange("p k c -> p (k c)")  # [128, 8192]
        for h in (16, 8, 4, 2):
            w = h * COUT
            nc.vector.tensor_tensor(
                out=dv[:, :w], in0=dv[:, :w], in1=dv[:, w:2 * w], op=mx
            )
        g, r = divmod(t, STG)
        if r == 0:
            n_in_g = min(STG, MT - g * STG)
            stage2 = out_pool.tile([128, STG, COUT], FP32)
        # final level + f32 upcast
        nc.vector.tensor_tensor(
            out=stage2[:, r, :], in0=dv[:, :COUT], in1=dv[:, COUT:2 * COUT], op=mx
        )
        if r == n_in_g - 1:
            eng = nc.sync if g % 2 == 0 else nc.scalar
            eng.dma_start(
                out=outT_v[g * STG:g * STG + n_in_g].rearrange("t p c -> p t c"),
                in_=stage2[:, :n_in_g, :],
            )


def build_program():
    nc = bacc.Bacc("TRN2", target_bir_lowering=False, debug=False,
                   num_swdge_queues=NQ)
    xin = nc.dram_tensor("xin", [CIN + 3, NPAD], FP32, kind="ExternalInput")
    w1t = nc.dram_tensor("w1t", [CIN + 3, CMID], FP32, kind="ExternalInput")
    w2t = nc.dram_tensor("w2t", [CMID, COUT], FP32, kind="ExternalInput")
    b1c = nc.dram_tensor("b1c", [CMID, 1], FP32, kind="ExternalInput")
    idxt = nc.dram_tensor(
        "idxt", [128, MT * NW], mybir.dt.int16, kind="ExternalInput"
    )
    outT = nc.dram_tensor("outT", [MPAD, COUT], FP32, kind="ExternalOutput")
    x2t = nc.dram_tensor("x2t", [NPAD, COUT], BF16, kind="Internal")

    from contextlib import ExitStack

    with tile.TileContext(nc) as tc:
        with ExitStack() as ctx:
            build_body(ctx, tc, xin, w1t, w2t, b1c, idxt, outT, x2t)
    nc.compile()
    return nc


def make_in_map(features_b, xyz_b, idx_b, W1, W2, b1):
    """Build one core's input map from one batch's raw inputs (numpy)."""
    xin = np.zeros((CIN + 3, NPAD), np.float32)
    xin[:CIN, :N] = features_b
    xin[CIN:CIN + 3, :N] = xyz_b.T
    idxp = np.zeros((MPAD, K), np.int16)
    idxp[:M] = idx_b.astype(np.int16)
    # per call t the index list is i = k*128 + ml -> neighbor[t*128+ml, k]
    # (so index i lands in dst partition i%128 = ml, slot i//128 = k),
    # wrapped into 16 partitions (idx i at [i%16, i//16]) and replicated
    # across the 8 gpsimd cores' partition groups.
    lst = idxp.reshape(MT, 128, K).transpose(0, 2, 1).reshape(MT, NI)
    w = lst.reshape(MT, NW, 16).transpose(0, 2, 1)      # [MT, 16, NW]
    w = np.tile(w, (1, 8, 1))                           # [MT, 128, NW]
    idxt = np.ascontiguousarray(
        w.transpose(1, 0, 2).reshape(128, MT * NW).astype(np.int16)
    )
    return {
        "xin": xin,
        "w1t": np.ascontiguousarray(W1.T.astype(np.float32)),
        "w2t": np.ascontiguousarray(W2.T.astype(np.float32)),
        "b1c": np.ascontiguousarray(b1.astype(np.float32).reshape(CMID, 1)),
        "idxt": idxt,
    }


def postprocess(outT_np, b2):
    """[MPAD, 256] device result -> [256, 6000] final (+b2, relu, T)."""
    o = outT_np[:M].astype(np.float32) + b2.astype(np.float32)[None, :]
    np.maximum(o, 0.0, out=o)
    return np.ascontiguousarray(o.T)


def run(inputs, trace=False, **spmd_kwargs):
    """Run on 8 NeuronCores; returns (out [8,256,6000] f32, BassKernelResults)."""
    features = np.asarray(inputs["features"], np.float32)
    support_xyz = np.asarray(inputs["support_xyz"], np.float32)
    neighbor_idx = np.asarray(inputs["neighbor_idx"])
    W1 = np.asarray(inputs["W1"], np.float32)
    W2 = np.asarray(inputs["W2"], np.float32)
    b1 = np.asarray(inputs["b1"], np.float32)
    b2 = np.asarray(inputs["b2"], np.float32)

    if "nc" not in _CACHE:
        _CACHE["nc"] = build_program()
    nc = _CACHE["nc"]

    in_maps = [
        make_in_map(features[b], support_xyz[b], neighbor_idx[b], W1, W2, b1)
        for b in range(B)
    ]
    res = run_bass_kernel_spmd(
        nc, in_maps, core_ids=list(range(B)), trace=trace, **spmd_kwargs
    )
    out = np.stack(
        [postprocess(res.results[b]["outT"], b2) for b in range(B)]
    ).astype(np.float32)
    return out, res


def kernel(query_xyz, support_xyz, features, neighbor_idx, W1, b1, W2, b2,
           **unused):
    del query_xyz  # neighborhoods are precomputed; query coords unused
    out, _ = run(
        dict(
            support_xyz=support_xyz,
            features=features,
            neighbor_idx=neighbor_idx,
            W1=W1,
            b1=b1,
            W2=W2,
            b2=b2,
        )
    )
    return out


if __name__ == "__main__":
    nc = build_program()
    print("program built ok")
